# revision 39
# baseline (speedup 1.0000x reference)
"""AdditiveAttention on 8 TRN2 NeuronCores.

Math: out = softmax_k(mask(sum_h w_v[h] * tanh(qp[b,q,h] + kp[b,k,h]))) @ values
with qp = queries @ W_q^T, kp = keys @ W_k^T, mask from valid_lens (B,).

Key idea: tanh(q+k) is approximated by a sparse harmonic sine series
    tanh(u) ~= sum_{r in RS} b_r sin(r*w0*u),   RS = [1,2,3,4,5,6,8,10]
which FACTORIZES via angle addition:
    sin(r*w0*(q+k)) = sin(r*w0*q)cos(r*w0*k) + cos(r*w0*q)sin(r*w0*k)
so the (B,Q,K,H) tanh tensor never materializes: the per-element work is
O((Q+K)*H*R) sin/cos harmonics (ScalarE base + VectorE recurrences) and the
(q,k) coupling is TensorE matmuls with contraction over (h, r, trig).

Harmonics: base sin/cos(w0*x) from ScalarE Sin (args kept inside [-pi,pi] by
choice of w0); odd r via step-2 Chebyshev recurrence S[r+2]=2cos(2th)S[r]-S[r-2]
(VectorE, fp16, 2x mode); even r=2m via doubling sin2m = s_m*c_m (stored scaled
by 1/2^a) and cos2m = 1 - 2*4^a*s_m^2 (ScalarE affine). All scale compensations
and w_v/b_r folding happen in per-partition ACT scale columns computed on host.

Masking: keys are truncated/padded to KP (multiple of 128) >= max(valid_lens);
a rank-1 matmul row adds -60000 to padded score columns, so exp underflows to
exactly 0 like the reference's -1e6 mask.

Sharding: core c handles batch c//2, query rows (c%2)*256..+256.

w0 and the fit coefficients b_r are computed from the actual inputs at call
time (host-side range analysis + least squares), then baked into the program.
"""

import math
from contextlib import ExitStack

import numpy as np

import concourse.bass as bass
import concourse.mybir as mybir
import concourse.tile as tile
from concourse import bacc
from concourse.bass_utils import run_bass_kernel_spmd

B, Q, K, D, H, V = 4, 512, 512, 256, 256, 256
NCORES = 8
NQ = (B * Q) // NCORES          # 256 query rows per core
# sparse harmonic set: odds need the (expensive) step-2 chain, evens come
# cheap from doubling, so high harmonics are even-only
RS = [1, 2, 3, 4, 5, 6, 8, 10]
NR = len(RS)
NEGM = -60000.0                 # mask add (fits fp16; exp -> exactly 0 in fp32)
FP32 = mybir.dt.float32
FP16 = mybir.dt.float16
AX = mybir.AxisListType
ALU = mybir.AluOpType
ACTF = mybir.ActivationFunctionType

# stored sin_r = sin_r / 2^{A_EXP[r]} from the doubling scheme
A_EXP = {r: 0 for r in RS if r % 2 == 1}
for _r in sorted(r for r in RS if r % 2 == 0):
    A_EXP[_r] = A_EXP[_r // 2] + 1
RS_SCHED = list(RS)
assert sorted(RS_SCHED) == sorted(RS)
EVENS = [r for r in RS if r % 2 == 0]
EXACT_EVENS = [r for r in EVENS if 2 * r in RS]
TILDE_EVENS = [r for r in EVENS if 2 * r not in RS]
ODDS = sorted(r for r in RS if r % 2 == 1 and r >= 3)
for _r in EVENS:
    assert _r // 2 in RS, f"doubling source {_r//2} missing for harmonic {_r}"


def fit_series(qp, kp, vls):
    """Range analysis + least-squares harmonic fit. qp/kp: [b][h, *]."""
    umax, xmax = 0.0, 0.0
    for b in range(B):
        kv = kp[b][:, : vls[b]]
        umax = max(umax, (qp[b].max(1) + kv.max(1)).max(),
                   -(qp[b].min(1) + kv.min(1)).min())
        xmax = max(xmax, np.abs(qp[b]).max(), np.abs(kv).max())
    P = max(2.0 * (umax + 0.15), 4.0 * xmax + 0.08)
    w0 = 2.0 * np.pi / P
    u = np.linspace(-(umax + 0.05), umax + 0.05, 4001)
    A = np.stack([np.sin(r * w0 * u) for r in RS], 1)
    wgt = np.exp(-(u ** 2) / (2 * 2.5 ** 2)) + 1e-3
    sw = np.sqrt(wgt)[:, None]
    bco, *_ = np.linalg.lstsq(A * sw, np.tanh(u) * sw[:, 0], rcond=None)
    return float(w0), bco.astype(np.float64)


def pack_layout(KP):
    """Column offsets of the single packed (128, X) fp16 input tile."""
    NK = KP // 128
    names = ([("wq0", H), ("wq1", H), ("qT0", NQ), ("qT1", NQ),
              ("wk0", H), ("wk1", H), ("kT0", KP), ("kT1", KP)]
             + [(f"v{i}", V) for i in range(NK)] + [("ident", 128)])
    off, x = {}, 0
    for nm, w in names:
        off[nm] = x
        x += w
    return off, x, (off["wk0"], off["v0"])


def build_nc(w0, bco, KP):
    """Build the SPMD Bass program. KP = padded key length (multiple of 128)."""
    NK = KP // 128
    QW = 2 * NQ                  # combined q-part width (2 h-chunks)
    CW = QW + 2 * KP             # combined tile width: [hc0 q | hc1 q | hc0 k | hc1 k]
    OFF, PX, CUT = pack_layout(KP)

    nc = bacc.Bacc()
    pack = nc.declare_dram_parameter("pack", [128, PX], FP16, isOutput=False)
    mo = nc.declare_dram_parameter("mo", [1, KP + 128], FP16, isOutput=False)
    cols = nc.declare_dram_parameter("cols", [128, 2 * NR + 2 * len(EVENS)], FP32,
                                     isOutput=False)
    out_d = nc.declare_dram_parameter("out", [NQ, V], FP32, isOutput=True)

    def colA(t, hc, r):
        j = hc * NR + RS.index(r)
        return t[:, j: j + 1]

    def colB(t, hc, r):
        j = 2 * NR + hc * len(EVENS) + EVENS.index(r)
        return t[:, j: j + 1]

    with TileCtx(nc) as (tc, ctx):
        inp = ctx.enter_context(tc.tile_pool(name="inp", bufs=1))
        harm = ctx.enter_context(tc.tile_pool(name="harm", bufs=1))
        qbp = ctx.enter_context(tc.tile_pool(name="qb", bufs=1))
        tmp_pool = ctx.enter_context(tc.tile_pool(name="tmp", bufs=3))
        sm = ctx.enter_context(tc.tile_pool(name="sm", bufs=1))
        ps_big = ctx.enter_context(tc.tile_pool(name="psA", bufs=1, space="PSUM"))
        ps_sc = ctx.enter_context(tc.tile_pool(name="psS", bufs=1, space="PSUM"))
        ps_pt = ctx.enter_context(tc.tile_pool(name="psT", bufs=2, space="PSUM"))

        # ---- input DMAs: one packed tile + mask/ones row ----
        mo_sb = inp.tile([1, KP + 128], FP16, tag="mo", name="mo_sb")
        nc.sync.dma_start(out=mo_sb, in_=mo[:, :])
        big = inp.tile([128, PX], FP16, tag="big", name="big")
        c1_, c2_ = CUT
        nc.sync.dma_start(out=big[:, :c1_], in_=pack[:, :c1_])
        nc.sync.dma_start(out=big[:, c1_:c2_], in_=pack[:, c1_:c2_])
        nc.sync.dma_start(out=big[:, c2_:], in_=pack[:, c2_:])
        warm = inp.tile([1, 128], FP16, tag="warm", name="warm")
        # table-load warmup: make Sin tables resident while the big DMA runs
        nc.scalar.activation(warm, mo_sb[:, KP: KP + 128], ACTF.Sin, scale=0.001)

        qT_sb = [big[:, OFF[f"qT{i}"]: OFF[f"qT{i}"] + NQ] for i in range(2)]
        kT_sb = [big[:, OFF[f"kT{i}"]: OFF[f"kT{i}"] + KP] for i in range(2)]
        wq_sb = [big[:, OFF[f"wq{i}"]: OFF[f"wq{i}"] + H] for i in range(2)]
        wk_sb = [big[:, OFF[f"wk{i}"]: OFF[f"wk{i}"] + H] for i in range(2)]
        v_sb = [big[:, OFF[f"v{i}"]: OFF[f"v{i}"] + V] for i in range(NK)]
        mrow_sb = mo_sb[:, :KP]
        ones_r = mo_sb[:, KP: KP + 128]
        ident = big[:, OFF["ident"]: OFF["ident"] + 128]
        cols_sb = inp.tile([128, 2 * NR + 2 * len(EVENS)], FP32, tag="cols", name="cols_sb")
        nc.sync.dma_start(out=cols_sb, in_=cols[:, :])
        hpi = inp.tile([128, 1], FP32, tag="hpi", name="hpi")
        nc.gpsimd.memset(hpi, math.pi / 2)

        # ---- projections (fp32): qp/kp [h=128 x hc, *] in PSUM ----
        qp_ps = [ps_big.tile([128, NQ], FP32, tag=f"bigA{hc}", name=f"qp{hc}")
                 for hc in range(2)]
        kp_ps = [ps_big.tile([128, KP], FP32, tag=f"bigK{hc}", name=f"kp{hc}")
                 for hc in range(2)]
        for hc in range(2):
            for dc in range(2):
                nc.tensor.matmul(qp_ps[hc], wq_sb[dc][:, 128 * hc: 128 * (hc + 1)],
                                 qT_sb[dc], start=(dc == 0), stop=(dc == 1))
        for hc in range(2):
            for dc in range(2):
                nc.tensor.matmul(kp_ps[hc], wk_sb[dc][:, 128 * hc: 128 * (hc + 1)],
                                 kT_sb[dc], start=(dc == 0), stop=(dc == 1))

        # ---- base harmonics r=1 (ScalarE Sin; args within [-pi, pi]) ----
        sc = {r: harm.tile([128, 2, CW], FP16, tag=f"sc{r}", name=f"sc{r}") for r in RS}
        s = {r: sc[r][:, 0] for r in RS}
        c = {r: sc[r][:, 1] for r in RS}
        sqm_needed = [1] + [r // 2 for r in EXACT_EVENS if r // 2 != 1]
        sq = {m: harm.tile([128, CW], FP16, tag=f"sq{m}", name=f"sq{m}") for m in sqm_needed}
        m2 = harm.tile([128, CW], FP16, tag="m2", name="m2")
        HPI = math.pi / 2
        for hc in range(2):
            qsl = slice(hc * NQ, (hc + 1) * NQ)
            ksl = slice(QW + hc * KP, QW + (hc + 1) * KP)
            nc.scalar.activation(s[1][:, qsl], qp_ps[hc], ACTF.Sin, scale=w0)
            nc.scalar.activation(s[1][:, ksl], kp_ps[hc], ACTF.Sin, scale=w0)
        for hc in range(2):
            qsl = slice(hc * NQ, (hc + 1) * NQ)
            ksl = slice(QW + hc * KP, QW + (hc + 1) * KP)
            nc.scalar.activation(c[1][:, qsl], qp_ps[hc], ACTF.Sin, scale=w0, bias=hpi)
            nc.scalar.activation(c[1][:, ksl], kp_ps[hc], ACTF.Sin, scale=w0, bias=hpi)
        # switch ACT tables to the exp set now, hidden under DVE/PE work,
        # so the softmax exp at the tail doesn't pay the ~1.3us load
        nc.scalar.activation(warm, warm, ACTF.Exp)

        # ---- harmonics r=2..R on DVE, r-ascending for pipelining.
        # odd chain: S[r+2] = m2*S[r] - S[r-2] (S[1]=-S[-1], C[-1]=C[1]);
        # even doubling: s[2m] = s[m]*c[m] (scaled 1/2^a); c[2m] affine of s[m]^2
        # (fused 2-op tensor_scalar on DVE).
        nc.vector.tensor_mul(sq[1], s[1], s[1])
        nc.vector.tensor_mul(s[2], s[1], c[1])
        nc.vector.tensor_scalar(c[2], sq[1], -2.0, 1.0, ALU.mult, ALU.add)
        nc.vector.tensor_scalar(m2, sq[1], -4.0, 2.0, ALU.mult, ALU.add)
        t0 = tmp_pool.tile([128, CW], FP16, tag="tmp", name="tmp")
        nc.vector.tensor_mul(t0, m2, s[1])
        nc.vector.tensor_add(s[3], t0, s[1])
        t1 = tmp_pool.tile([128, CW], FP16, tag="tmp", name="tmp")
        nc.vector.tensor_mul(t1, m2, c[1])
        nc.vector.tensor_sub(c[3], t1, c[1])
        for r in [r for r in RS_SCHED if r >= 4]:
            if r % 2 == 0:
                m = r // 2
                nc.vector.tensor_mul(s[r], s[m], c[m])
                if r in EXACT_EVENS:
                    nc.vector.tensor_mul(sq[m], s[m], s[m])
                    nc.vector.tensor_scalar(c[r], sq[m], -2.0 * (4.0 ** A_EXP[m]),
                                            1.0, ALU.mult, ALU.add)
                else:
                    # ctilde = cos-1 in one fused op; the dropped +1 is a
                    # per-q-row score constant, invisible to softmax
                    nc.vector.scalar_tensor_tensor(
                        c[r], s[m], -2.0 * (4.0 ** A_EXP[m]), s[m],
                        ALU.mult, ALU.mult)
            else:
                ta = tmp_pool.tile([128, CW], FP16, tag="tmp", name="tmp")
                nc.vector.tensor_mul(ta, m2, s[r - 2])
                nc.vector.tensor_sub(s[r], ta, s[r - 4])
                tb = tmp_pool.tile([128, CW], FP16, tag="tmp", name="tmp")
                nc.vector.tensor_mul(tb, m2, c[r - 2])
                nc.vector.tensor_sub(c[r], tb, c[r - 4])

        # ---- q-side b-scaled stationaries (ScalarE Copy, per-partition cols)
        SCb = {r: qbp.tile([128, 2, QW], FP16, tag=f"SCb{r}", name=f"SCb{r}")
               for r in RS}
        Sqb = {r: SCb[r][:, 0] for r in RS}
        Cqb = {r: SCb[r][:, 1] for r in RS}
        for r in RS_SCHED:
            for hc in range(2):
                qsl = slice(hc * NQ, (hc + 1) * NQ)
                if r % 2 == 1:
                    # fused: both trig halves in one strided-AP pass
                    nc.scalar.activation(SCb[r][:, :, qsl], sc[r][:, :, qsl],
                                         ACTF.Copy, scale=colA(cols_sb, hc, r))
                elif r in EXACT_EVENS:
                    nc.scalar.activation(Sqb[r][:, qsl], s[r][:, qsl], ACTF.Copy,
                                         scale=colA(cols_sb, hc, r))
                    nc.scalar.activation(Cqb[r][:, qsl], sq[r // 2][:, qsl],
                                         ACTF.Identity,
                                         scale=colB(cols_sb, hc, r),
                                         bias=colA(cols_sb, hc, r))
                else:
                    nc.scalar.activation(Sqb[r][:, qsl], s[r][:, qsl], ACTF.Copy,
                                         scale=colA(cols_sb, hc, r))
                    nc.scalar.activation(Cqb[r][:, qsl], c[r][:, qsl],
                                         ACTF.Identity,
                                         scale=colA(cols_sb, hc, r),
                                         bias=colA(cols_sb, hc, r))

        # ---- scores: PSUM accumulation of 4R+1 matmuls per q-tile ----
        sc_ps = []
        for qt in range(2):
            sc = ps_sc.tile([128, KP], FP32, tag=f"sc{qt}", name=f"sc{qt}")
            sc_ps.append(sc)
            # mask row first: PE starts right after the small input DMA
            nc.tensor.matmul(sc, ones_r, mrow_sb, start=True, stop=False)
            for r in RS_SCHED:
                for hc in range(2):
                    qsl = slice(hc * NQ + qt * 128, hc * NQ + (qt + 1) * 128)
                    ksl = slice(QW + hc * KP, QW + (hc + 1) * KP)
                    nc.tensor.matmul(sc, Sqb[r][:, qsl], c[r][:, ksl],
                                     start=False, stop=False)
                    nc.tensor.matmul(sc, Cqb[r][:, qsl], s[r][:, ksl],
                                     start=False,
                                     stop=(r == RS_SCHED[-1] and hc == 1))

        # ---- softmax + AV per q-tile ----
        for qt in range(2):
            sc = sc_ps[qt]
            negmax = sm.tile([128, 1], FP32, tag=f"nm{qt}", name=f"nm{qt}")
            nc.vector.reduce_max(negmax, sc, axis=AX.X, negate=True)
            p_sb = sm.tile([128, KP], FP16, tag=f"p{qt}", name=f"p{qt}")
            ssum = sm.tile([128, 1], FP32, tag=f"ss{qt}", name=f"ss{qt}")
            nc.scalar.activation(p_sb, sc, ACTF.Exp, bias=negmax, accum_out=ssum)
            rs = sm.tile([128, 1], FP32, tag=f"rs{qt}", name=f"rs{qt}")
            nc.vector.reciprocal(rs, ssum)

            av = ps_big.tile([128, V], FP32, tag=f"bigA{qt}", name=f"av{qt}")
            for kc in range(NK):
                ptp = ps_pt.tile([128, 128], FP16, tag="pt", name="pt")
                nc.tensor.transpose(ptp, p_sb[:, 128 * kc: 128 * (kc + 1)], ident)
                pts = tmp_pool.tile([128, 128], FP16, tag="pts", name="pts")
                nc.vector.tensor_copy(pts, ptp)
                nc.tensor.matmul(av, pts, v_sb[kc],
                                 start=(kc == 0), stop=(kc == NK - 1))
            o_sb = sm.tile([128, V], FP32, tag=f"o{qt}", name=f"o{qt}")
            nc.scalar.activation(o_sb, av, ACTF.Copy, scale=rs)
            nc.sync.dma_start(out=out_d[128 * qt: 128 * (qt + 1), :], in_=o_sb)

    nc.compile()
    return nc


class TileCtx:
    """TileContext + ExitStack in one `with`."""

    def __init__(self, nc):
        self.nc = nc

    def __enter__(self):
        self.ctx = ExitStack()
        self.tc = self.ctx.enter_context(tile.TileContext(self.nc))
        return self.tc, self.ctx

    def __exit__(self, *exc):
        return self.ctx.__exit__(*exc)


def prepare(inputs):
    """Host prep: shards, fit, per-core input maps."""
    queries = np.ascontiguousarray(np.asarray(inputs["queries"], np.float32))
    keys = np.ascontiguousarray(np.asarray(inputs["keys"], np.float32))
    values = np.ascontiguousarray(np.asarray(inputs["values"], np.float32))
    vls = np.asarray(inputs["valid_lens"]).astype(np.int64)
    Wq = np.asarray(inputs["W_q"], np.float32)
    Wk = np.asarray(inputs["W_k"], np.float32)
    wv = np.asarray(inputs["w_v"], np.float32)

    # device projections run on fp16-rounded inputs; match that for ranges
    q16 = queries.astype(np.float16).astype(np.float32)
    k16 = keys.astype(np.float16).astype(np.float32)
    Wq16 = Wq.astype(np.float16).astype(np.float32)
    Wk16 = Wk.astype(np.float16).astype(np.float32)
    qp = [(Wq16 @ q16[b].T).astype(np.float32) for b in range(B)]   # [h, q]
    kp = [(Wk16 @ k16[b].T).astype(np.float32) for b in range(B)]   # [h, k]
    w0, bco = fit_series(qp, kp, vls)
    KP = 128 * max(1, int(math.ceil(vls.max() / 128.0)))

    # per-partition scale columns (same for every core)
    ncolb = len(EVENS)
    cols = np.zeros((128, 2 * NR + 2 * ncolb), np.float32)
    for hc in range(2):
        wvh = wv[128 * hc: 128 * (hc + 1)]
        for j, r in enumerate(RS):
            cols[:, hc * NR + j] = wvh * bco[j] * (2.0 ** A_EXP[r])
        for j, r in enumerate(EVENS):
            cols[:, 2 * NR + hc * ncolb + j] = (
                -2.0 * (4.0 ** A_EXP[r // 2]) * wvh * bco[RS.index(r)]
                * (2.0 ** A_EXP[r]))

    OFF, PX, _CUT = pack_layout(KP)
    NK = KP // 128
    in_maps = []
    for core in range(NCORES):
        b, qlo = core // 2, (core % 2) * NQ
        n = int(vls[b])
        pk = np.zeros((128, PX), np.float16)
        qTm = queries[b, qlo: qlo + NQ].T.astype(np.float16)        # (D, NQ)
        kTm = np.zeros((D, KP), np.float16)
        kTm[:, :n] = keys[b, :n].T.astype(np.float16)
        for i in range(2):
            pk[:, OFF[f"qT{i}"]: OFF[f"qT{i}"] + NQ] = qTm[128 * i: 128 * (i + 1)]
            pk[:, OFF[f"kT{i}"]: OFF[f"kT{i}"] + KP] = kTm[128 * i: 128 * (i + 1)]
            pk[:, OFF[f"wq{i}"]: OFF[f"wq{i}"] + H] = Wq.T[128 * i: 128 * (i + 1)].astype(np.float16)
            pk[:, OFF[f"wk{i}"]: OFF[f"wk{i}"] + H] = Wk.T[128 * i: 128 * (i + 1)].astype(np.float16)
        vm = np.zeros((KP, V), np.float16)
        vm[:n] = values[b, :n].astype(np.float16)
        for i in range(NK):
            pk[:, OFF[f"v{i}"]: OFF[f"v{i}"] + V] = vm[128 * i: 128 * (i + 1)]
        pk[:, OFF["ident"]: OFF["ident"] + 128] = np.eye(128, dtype=np.float16)
        mov = np.zeros((1, KP + 128), np.float16)
        mov[0, :KP] = np.where(np.arange(KP) < n, 0.0, NEGM).astype(np.float16)
        mov[0, KP:] = 1.0
        in_maps.append({"pack": pk, "mo": mov, "cols": cols})
    return w0, bco, KP, in_maps


def kernel(**inputs):
    w0, bco, KP, in_maps = prepare(inputs)
    nc = build_nc(w0, bco, KP)
    res = run_bass_kernel_spmd(nc, in_maps, core_ids=list(range(NCORES)))
    out = np.zeros((B, Q, V), np.float32)
    for core in range(NCORES):
        b, qlo = core // 2, (core % 2) * NQ
        out[b, qlo: qlo + NQ] = res.results[core]["out"]
    return out
